# revision 25
# baseline (speedup 1.0000x reference)
"""GQA kernel for Trainium2: B=2,T=2048,E=2048,G=4,QPG=4,D=128, causal + sinusoidal PE.

Sharding: one core per (batch, kv-group) pair = 2*4 = 8 cores.
Each core computes q/k/v projections for its group, attention for its 4 query
heads, and a partial output projection (its group's 512 columns of wo);
partials are summed on the host.

v3 layout strategy (per core), all matmul operands bf16:
  - wqkv columns are host-reordered to [q0,q1,k,v,q2,q3] and DMA'd in two
    halves so everything pair (0,1) needs lands first; block-0's four
    critical chains (q0,q1,k,v) are interleaved e-outer so the PE consumes
    tiles at DMA delivery rate instead of stalling chain-by-chain.
  - projections: weight tile stationary, x^T streams; per (tb, chain) one
    512-col psum chain over 16 e-tiles. pe^T+bias folded into a DVE add.
  - scores S^T[tk, tq]: kt tile stationary, q^T streams; exp'd on ACT into
    bf16 P^T tiles. Diagonal tiles are column-restricted to the unmasked
    range [128j, 512) for the matmul, exp, mask, R-add and PV.
  - PV: V tile [tk, d] stationary, P^T streams, O^T[d, tq] in psum.
  - softmax denominators: R = sum_tk P^T accumulated on DVE, then ONE PE
    matmul against an all-ones stationary matrix replicates the column sums
    to all partitions (replaces the 3.6us gpsimd partition_all_reduce that
    head-of-line blocked the DVE queue at block boundaries); reciprocal
    multiplies O^T during the psum drain.
  - phases are software-pipelined: proj half-chains of block tb+1 and
    output-projection units of block qb-1 are interleaved into the
    ACT-paced scores stretch one unit per accumulated lag quantum.
  - tail: the last block's output stores go out per-512-col chunk right
    after each chunk's psum drain.
"""
import sys

sys.path.insert(0, "/opt/trn_rl_repo")

import math
import numpy as np

B, T, E = 2, 2048, 2048
G, QPG, D = 4, 4, 128
NQ = QPG * D          # 512 q columns per group
NKV = 2 * D           # 256 kv columns per group
TT = T // 128         # 16 t-tiles
TB = T // 512         # 4 t-blocks
NE = E // 128         # 16 e-tiles
ISD = 1.0 / math.sqrt(D)

# wqkv column order: [q0 | q1 | k | v | q2 | q3]
WCOL = {0: 0, 1: 128, 4: 256, 5: 384, 2: 512, 3: 640}

_compiled = None


def _build():
    from collections import deque
    from concourse import bacc, tile, mybir

    f32 = mybir.dt.float32
    bf16 = mybir.dt.bfloat16
    ADD = mybir.AluOpType.add
    MULT = mybir.AluOpType.mult
    EXP = mybir.ActivationFunctionType.Exp

    nc = bacc.Bacc("TRN2", target_bir_lowering=False, debug=False, num_devices=8)

    xt_d = nc.dram_tensor("xt", [128, NE, T], bf16, kind="ExternalInput")    # x^T packed
    wqkv_d = nc.dram_tensor("wqkv", [128, NE, NQ + NKV], bf16, kind="ExternalInput")
    wo_d = nc.dram_tensor("wo", [128, QPG, E], bf16, kind="ExternalInput")  # group slice, head-major
    pet_d = nc.dram_tensor("pet", [D, T], bf16, kind="ExternalInput")         # pe^T
    bq_d = nc.dram_tensor("bq", [D, QPG], f32, kind="ExternalInput")
    bk_d = nc.dram_tensor("bk", [D, 1], f32, kind="ExternalInput")
    bv_d = nc.dram_tensor("bv", [D, 1], f32, kind="ExternalInput")
    msk_d = nc.dram_tensor("msk", [128, 4, 512], bf16, kind="ExternalInput")
    idb_d = nc.dram_tensor("idb", [128, 128], bf16, kind="ExternalInput")
    ones_d = nc.dram_tensor("ones", [128, 128], bf16, kind="ExternalInput")
    out_d = nc.dram_tensor("out", [T, E], bf16, kind="ExternalOutput")

    with tile.TileContext(nc) as tc:
        with (
            tc.tile_pool(name="main", bufs=1) as pp,
            tc.tile_pool(name="ps", bufs=1, space="PSUM") as ps,
        ):
            # ---- the whole startup is HBM-bandwidth-bound (~270 GB/s
            # aggregate) and per-queue FIFOs share bandwidth unfairly, so
            # everything flows through TWO balanced queues (sync/scalar) in
            # strict deadline order; gpsimd only carries tiny constants ----
            bq = pp.tile([D, QPG], f32)
            nc.gpsimd.dma_start(bq[:], bq_d[:])
            bk = pp.tile([D, 1], f32)
            nc.gpsimd.dma_start(bk[:], bk_d[:])
            bv = pp.tile([D, 1], f32)
            nc.gpsimd.dma_start(bv[:], bv_d[:])
            idb = pp.tile([128, 128], bf16)
            nc.gpsimd.dma_start(idb[:], idb_d[:])
            ones = pp.tile([128, 128], bf16)
            nc.gpsimd.dma_start(ones[:], ones_d[:])
            pet_t = [pp.tile([D, 512], bf16, name=f"pet{tb}", tag=f"pet{tb}")
                     for tb in range(TB)]
            nc.scalar.dma_start(pet_t[0][:], pet_d[:, 0:512])
            msk01 = pp.tile([128, 2, 512], bf16, name="msk01", tag="msk01")
            msk23 = pp.tile([128, 2, 512], bf16, name="msk23", tag="msk23")
            wo_a = pp.tile([128, 2, E], bf16, name="wo_a", tag="wo_a")
            wo_b = pp.tile([128, 2, E], bf16, name="wo_b", tag="wo_b")

            # ---- persistent activations ----
            qt = [pp.tile([128, T], bf16, name=f"qt{h}", tag=f"qt{h}") for h in range(QPG)]
            kt = pp.tile([128, T], bf16)
            at = [pp.tile([128, T], bf16, name=f"at{h}", tag=f"at{h}") for h in range(QPG)]
            vxall = pp.tile([128, T], bf16, name="vxall", tag="vxall")
            # pe^T with bias pre-folded per head, built on DVE while it idles
            # during the phase-1 stream: drains become a single DVE add
            petq = [pp.tile([128, T], bf16, name=f"petq{h}", tag=f"petq{h}")
                    for h in range(QPG)]
            petk = pp.tile([128, T], bf16, name="petk", tag="petk")

            # ---- phase-1 weights + x^T stream; the [q0,q1,k,v] and [q2,q3]
            # column halves live in separate tiles so the late second-half
            # DMA can never create a false hazard against first-half reads ----
            wqkv_sb = [pp.tile([128, 4, 512], bf16, name=f"wqkv{g}", tag=f"wqkv{g}")
                       for g in range(4)]
            wqkv2_sb = [pp.tile([128, 4, 256], bf16, name=f"wqkv2_{g}", tag=f"wqkv2_{g}")
                        for g in range(4)]
            xt_t = [[None] * 4 for _ in range(TB)]

            def emit_xt_dma(tb):
                ts = slice(tb * 512, (tb + 1) * 512)
                for g in range(4):
                    xt_t[tb][g] = pp.tile([128, 4, 512], bf16, name="xt", tag="xt", bufs=8)
                    eng = nc.sync if g % 2 == 0 else nc.scalar
                    eng.dma_start(xt_t[tb][g][:], xt_d[:, 4 * g:4 * g + 4, ts])

            # block-0 feed: per-e (w,x) round-robin over all three DMA
            # queues (two queues are issue-rate-bound at ~650ns per dma_start)
            for g in range(4):
                xt_t[0][g] = pp.tile([128, 4, 512], bf16, name="xt", tag="xt", bufs=8)
            qs3 = [nc.sync, nc.scalar, nc.gpsimd]
            for e in range(NE):
                g, i = divmod(e, 4)
                qs3[(2 * e) % 3].dma_start(wqkv_sb[g][:, i, :], wqkv_d[:, e, 0:512])
                qs3[(2 * e + 1) % 3].dma_start(xt_t[0][g][:, i, :], xt_d[:, e, 0:512])
                if e == 8:
                    nc.sync.dma_start(pet_t[1][:], pet_d[:, 512:1024])
            # balanced deadline-ordered tails on sync+scalar only (gpsimd
            # must go idle so it can't out-arbitrate the e-stream): masks,
            # q2/q3 weight halves, remaining pet chunks, xt(1), wo halves
            nc.sync.dma_start(msk01[:], msk_d[:, 0:2, :])
            nc.scalar.dma_start(msk23[:], msk_d[:, 2:4, :])
            for g, eng in zip(range(4), (nc.sync, nc.scalar, nc.sync, nc.scalar)):
                eng.dma_start(wqkv2_sb[g][:], wqkv_d[:, 4 * g:4 * g + 4, 512:768])
            nc.sync.dma_start(pet_t[2][:], pet_d[:, 1024:1536])
            nc.scalar.dma_start(pet_t[3][:], pet_d[:, 1536:2048])
            emit_xt_dma(1)
            nc.sync.dma_start(wo_a[:], wo_d[:, 0:2, :])
            nc.scalar.dma_start(wo_b[:], wo_d[:, 2:4, :])

            def wsel(e, c):
                g, i = divmod(e, 4)
                if c in (2, 3):
                    return wqkv2_sb[g][:, i, (c - 2) * 128:(c - 1) * 128]
                return wqkv_sb[g][:, i, WCOL[c]:WCOL[c] + 128]

            def build_petx(tb):
                ts = slice(tb * 512, (tb + 1) * 512)
                for c in range(QPG):
                    nc.vector.tensor_tensor(petq[c][:, ts], pet_t[tb][:],
                                            bq[:, c:c + 1].to_broadcast([128, 512]), ADD)
                nc.vector.tensor_tensor(petk[:, ts], pet_t[tb][:],
                                        bk[:].to_broadcast([128, 512]), ADD)

            def v_bias(ts, ps_t):
                vtb = pp.tile([128, 512], bf16, name="vtb", tag="vtb", bufs=1)
                nc.vector.tensor_tensor(vtb[:], ps_t[:],
                                        bv[:].to_broadcast([128, 512]), ADD)
                return vtb

            def v_finish(ts, vtb):
                vtp = ps.tile([128, 512], bf16, name="vtp", tag="vtp", bufs=1)
                for i in range(4):
                    nc.tensor.transpose(vtp[:, i * 128:(i + 1) * 128],
                                        vtb[:, i * 128:(i + 1) * 128], idb[:])
                nc.vector.tensor_copy(vxall[:, ts], vtp[:])

            def proj_drain(c, ts, ps_t):
                if c < 4:
                    nc.vector.tensor_tensor(qt[c][:, ts], ps_t[:], petq[c][:, ts], ADD)
                elif c == 4:
                    nc.vector.tensor_tensor(kt[:, ts], ps_t[:], petk[:, ts], ADD)
                else:
                    v_finish(ts, v_bias(ts, ps_t))

            # ---- unit generators; units are (cost_ns, fn) ----
            def proj_units(tb, c):
                # two half-chain units + a drain emitted one unit late so the
                # completed psum never blocks masks/exps behind it on DVE
                ts = slice(tb * 512, (tb + 1) * 512)
                state = {}

                def half_a():
                    ps_t = ps.tile([128, 512], f32, name="big", tag="big", bufs=3)
                    state["ps"] = ps_t
                    for e in range(8):
                        nc.tensor.matmul(ps_t[:], wsel(e, c), xt_t[tb][e // 4][:, e % 4, :],
                                         start=(e == 0), stop=False)

                def half_b():
                    ps_t = state["ps"]
                    for e in range(8, NE):
                        nc.tensor.matmul(ps_t[:], wsel(e, c), xt_t[tb][e // 4][:, e % 4, :],
                                         start=False, stop=(e == NE - 1))

                def drain():
                    proj_drain(c, ts, state["ps"])

                return [(1800, half_a), (1800, half_b), (300, drain)]

            def oproj_units(ti, last=False):
                state = {}

                def alloc():
                    state["o"] = pp.tile([128, E], bf16, name="osb", tag="osb", bufs=2)

                state["pend"] = []

                def drain_oldest():
                    # drains run late and always on DVE: by emission time the
                    # psum is long finished, so neither the exp stream (ACT)
                    # nor the mask stream (DVE) ever waits behind it. Final
                    # block: store each chunk right after its drain so the
                    # last DMA starts as early as possible.
                    eo, w_ps = state["pend"].pop(0)
                    nc.vector.tensor_copy(state["o"][:, eo * 512:(eo + 1) * 512], w_ps[:])
                    if last:
                        eng = nc.sync if eo % 2 == 0 else nc.scalar
                        eng.dma_start(out_d[ti * 128:(ti + 1) * 128, eo * 512:(eo + 1) * 512],
                                      state["o"][:, eo * 512:(eo + 1) * 512])

                units = [(0, alloc)]
                for eo in range(4):
                    def one(eo=eo):
                        if len(state["pend"]) >= (1 if last else 2):
                            drain_oldest()
                        w_ps = ps.tile([128, 512], f32, name="big", tag="big", bufs=3)
                        for h in range(QPG):
                            nc.tensor.matmul(
                                w_ps[:], at[h][:, ti * 128:(ti + 1) * 128],
                                (wo_a if h < 2 else wo_b)[:, h % 2, eo * 512:(eo + 1) * 512],
                                start=(h == 0), stop=(h == QPG - 1),
                            )
                        state["pend"].append((eo, w_ps))
                    units.append((1000, one))

                def store():
                    while state["pend"]:
                        drain_oldest()
                    if not last:
                        eng = nc.scalar if ti % 2 else nc.sync
                        eng.dma_start(out_d[ti * 128:(ti + 1) * 128, :], state["o"][:])
                units.append((300, store))
                return units

            # two filler queues: proj has a deadline (before next block's
            # scores), oproj is slack-filled
            fill_proj = deque()
            fill_oproj = deque()
            lag = [0]

            def absorb(extra):
                lag[0] += extra
                while lag[0] > 0 and (fill_proj or fill_oproj):
                    q = fill_proj if fill_proj else fill_oproj
                    cost, fn = q.popleft()
                    fn()
                    lag[0] -= cost

            # ---- phase-2 per block qb, heads in pairs. Diagonal tiles are
            # column-restricted to [128j, 512). Denominators: DVE-accumulated
            # R, partition-summed by one ones-stationary matmul per head ----
            def phase2_pair(qb, pair):
                qs = slice(qb * 512, (qb + 1) * 512)
                nkt = 4 * qb + 4
                pts = {h: [] for h in pair}
                R = {h: pp.tile([128, 512], bf16, name=f"R{h}", tag=f"R{h}", bufs=1)
                     for h in pair}
                for tk in range(nkt):
                    j = tk - 4 * qb
                    lo = max(0, 128 * j)          # first unmasked col of block
                    w = 512 - lo
                    sl = slice(lo, 512)
                    for h in pair:
                        s_ps = ps.tile([128, 512], f32, name="s", tag="s", bufs=2)
                        nc.tensor.matmul(s_ps[:, sl], kt[:, tk * 128:(tk + 1) * 128],
                                         qt[h][:, qb * 512 + lo:qb * 512 + 512],
                                         start=True, stop=True)
                        p_t = pp.tile([128, 512], bf16, name="pt", tag="pt", bufs=32)
                        nc.scalar.activation(p_t[:, sl], s_ps[:, sl], EXP, scale=ISD)
                        if j >= 0:
                            mt = msk01[:, j, sl] if j < 2 else msk23[:, j - 2, sl]
                            nc.vector.tensor_tensor(p_t[:, sl], p_t[:, sl], mt, MULT)
                        pts[h].append((p_t, sl))
                        absorb(160 + (309 * w) // 512)
                o_ps = {h: ps.tile([128, 512], f32, name="o", tag="o", bufs=2)
                        for h in pair}
                for tk in range(nkt):
                    for h in pair:
                        p_t, sl = pts[h][tk]
                        nc.tensor.matmul(o_ps[h][:, sl], vxall[:, tk * 128:(tk + 1) * 128],
                                         p_t[:, sl], start=(tk == 0), stop=(tk == nkt - 1),
                                         skip_group_check=True)
                        # R accumulation rides the PV stretch, when DVE is
                        # otherwise idle
                        if tk == 0:
                            nc.vector.tensor_copy(R[h][:], p_t[:])
                        else:
                            nc.vector.tensor_tensor(R[h][:, sl], R[h][:, sl], p_t[:, sl], ADD)
                for h in pair:
                    rs = ps.tile([128, 512], f32, name="s", tag="s", bufs=2)
                    nc.tensor.matmul(rs[:], ones[:], R[h][:], start=True, stop=True)
                    r = pp.tile([128, 512], f32, name=f"rcp{h}", tag=f"rcp{h}", bufs=1)
                    nc.vector.reciprocal_approx_fast(r[:], rs[:])
                    nc.vector.tensor_tensor(at[h][:, qs], o_ps[h][:], r[:], MULT)

            # ---- drive ----
            # petq/petk for blocks 0 and 1 build on the idle DVE during the
            # phase-1 stream (their pet chunks arrive mid-stream, well before
            # the DVE reaches the drains behind them in its queue)
            build_petx(0)
            build_petx(1)
            # block-0 is DMA-paced: interleave the chains pair (0,1) needs
            # (q0, q1, k, v) e-outer so the PE tracks the DMA delivery
            # frontier; q2/q3 become fillers for pair (0,1)'s scores
            ch_ps = {}
            for c in (0, 1, 4):
                ch_ps[c] = ps.tile([128, 512], f32, name="big", tag="big", bufs=3)
            ch_ps[5] = ps.tile([128, 512], f32, name="s", tag="s", bufs=2)
            for e in range(NE):
                for c in (0, 1, 4, 5):
                    nc.tensor.matmul(ch_ps[c][:], wsel(e, c), xt_t[0][e // 4][:, e % 4, :],
                                     start=(e == 0), stop=(e == NE - 1))
            # drains in score-dependency order (q0, k, q1): the first scores
            # can start after two 1-op DVE adds. v gets its bias here (frees
            # the s-pool psum) but its transposes ride the first filler slot
            for c in (0, 4, 1):
                proj_drain(c, slice(0, 512), ch_ps[c])
            vtb0 = v_bias(slice(0, 512), ch_ps[5])

            def v_finish0():
                v_finish(slice(0, 512), vtb0)

            for qb in range(TB):
                if qb == 0:
                    tb0_rest = [(900, v_finish0)] + proj_units(0, 2) + proj_units(0, 3)
                    tb0_fns = {fn for _, fn in tb0_rest}
                    fill_proj.extend(tb0_rest)
                if qb + 1 < TB:
                    if qb > 0:
                        emit_xt_dma(qb + 1)   # xt(1) already issued up front
                        build_petx(qb + 1)
                    for c in range(6):
                        fill_proj.extend(proj_units(qb + 1, c))
                for pair in ((0, 1), (2, 3)):
                    if qb == 0 and pair == (2, 3):
                        # q2/q3 of block 0 must land before these scores
                        while fill_proj and fill_proj[0][1] in tb0_fns:
                            fill_proj.popleft()[1]()
                    phase2_pair(qb, pair)
                    if pair == (2, 3):
                        for ti in range(4 * qb, 4 * qb + 4):
                            fill_oproj.extend(oproj_units(ti, last=(qb == TB - 1)))
                # proj chains for tb=qb+1 must land before scores(qb+1)
                while fill_proj:
                    fill_proj.popleft()[1]()
            while fill_oproj:
                fill_oproj.popleft()[1]()

    nc.compile()
    return nc


def _get_compiled():
    global _compiled
    if _compiled is None:
        _compiled = _build()
    return _compiled


def _host_inputs(x, wq, bq, wkv, bkv, wo):
    import jax.numpy as jnp

    def to_bf16(a):
        return np.asarray(jnp.asarray(a, dtype=jnp.bfloat16))

    pos = np.arange(T, dtype=np.float32)[:, None]
    i = np.arange(0, D, 2, dtype=np.float32)
    inv = np.exp(-(np.log(10000.0) * i / D))
    ang = pos * inv
    pe = np.zeros((T, D), np.float32)
    pe[:, 0::2] = np.sin(ang)
    pe[:, 1::2] = np.cos(ang)
    pet = np.ascontiguousarray(pe.T)                       # [D, T]

    # causal masks for the 4 diagonal tiles of a 512-wide tq block:
    # mask_j[p, c] = 1 if c >= 128*j + p
    c = np.arange(512)[None, :]
    p = np.arange(128)[:, None]
    msk = to_bf16(np.ascontiguousarray(np.stack(
        [(c >= 128 * j + p) for j in range(4)]).astype(np.float32).transpose(1, 0, 2)))
    idb = to_bf16(np.eye(128, dtype=np.float32))
    ones = to_bf16(np.ones((128, 128), dtype=np.float32))

    xts = [to_bf16(np.ascontiguousarray(
        x[b].T.reshape(NE, 128, T).transpose(1, 0, 2))) for b in range(B)]
    in_maps = []
    for core in range(8):
        b, g = divmod(core, G)
        bq_g = bq[g * NQ:(g + 1) * NQ].reshape(QPG, D)     # [h, d]
        qh = [wq[:, g * NQ + h * 128:g * NQ + (h + 1) * 128] for h in range(QPG)]
        kcol = wkv[:, g * NKV:g * NKV + D]
        vcol = wkv[:, g * NKV + D:(g + 1) * NKV]
        # column order [q0, q1, k, v, q2, q3] to match WCOL
        wqkv = np.concatenate([qh[0], qh[1], kcol, vcol, qh[2], qh[3]], axis=1)
        in_maps.append({
            "xt": xts[b],
            "wqkv": to_bf16(np.ascontiguousarray(
                wqkv.reshape(NE, 128, NQ + NKV).transpose(1, 0, 2))),
            "wo": to_bf16(np.ascontiguousarray(
                wo[g * NQ:(g + 1) * NQ, :].reshape(QPG, 128, E).transpose(1, 0, 2))),
            "pet": to_bf16(pet),
            "bq": np.ascontiguousarray(bq_g.T).astype(np.float32),
            "bk": np.ascontiguousarray(
                bkv[g * NKV:g * NKV + D].reshape(D, 1)).astype(np.float32),
            "bv": np.ascontiguousarray(
                bkv[g * NKV + D:(g + 1) * NKV].reshape(D, 1)).astype(np.float32),
            "msk": msk,
            "idb": idb,
            "ones": ones,
        })
    return in_maps


def run(x, wq, bq, wkv, bkv, wo, trace=False):
    from concourse.bass_utils import run_bass_kernel_spmd

    nc = _get_compiled()
    in_maps = _host_inputs(
        np.asarray(x, np.float32), np.asarray(wq, np.float32),
        np.asarray(bq, np.float32), np.asarray(wkv, np.float32),
        np.asarray(bkv, np.float32), np.asarray(wo, np.float32),
    )
    res = run_bass_kernel_spmd(nc, in_maps, core_ids=list(range(8)), trace=trace)
    out = np.zeros((B, T, E), np.float32)
    for core in range(8):
        b = core // G
        out[b] += np.asarray(res.results[core]["out"], dtype=np.float32)
    return out, res


def kernel(x, wq, bq, wkv, bkv, wo):
    out, _ = run(x, wq, bq, wkv, bkv, wo, trace=False)
    return out


# revision 26
# speedup vs baseline: 1.0134x; 1.0134x over previous
"""GQA kernel for Trainium2: B=2,T=2048,E=2048,G=4,QPG=4,D=128, causal + sinusoidal PE.

Sharding: one core per (batch, kv-group) pair = 2*4 = 8 cores.
Each core computes q/k/v projections for its group, attention for its 4 query
heads, and a partial output projection (its group's 512 columns of wo);
partials are summed on the host.

v3 layout strategy (per core), all matmul operands bf16:
  - wqkv columns are host-reordered to [q0,q1,k,v,q2,q3] and DMA'd in two
    halves (separate tiles) so everything pair (0,1) needs lands first;
    block-0's four critical chains (q0,q1,k,v) are interleaved e-outer so
    the PE consumes tiles at DMA delivery rate.
  - projections: weight tile stationary, x^T streams; per (tb, chain) one
    512-col psum chain over 16 e-tiles.
  - scores S^T[tk, tq]: kt tile stationary, q^T streams; exp'd on ACT into
    bf16 P^T tiles. Diagonal tiles are column-restricted to the unmasked
    range [128j, 512) for the matmul, exp, mask, R-add and PV.
  - PV: V tile [tk, d] stationary, P^T streams, O^T[d, tq] in psum.
  - softmax denominators: R = sum_tk P^T accumulated on DVE, then ONE PE
    matmul against an all-ones stationary matrix replicates the column sums
    to all partitions (replaces the 3.6us gpsimd partition_all_reduce that
    head-of-line blocked the DVE queue at block boundaries); reciprocal
    multiplies O^T during the psum drain.
  - block-0 drains run in score-dependency order (q0, k, q1) and the
    v-transposes ride the first filler slot so the first scores start
    ~1.5us earlier.
  - phases are software-pipelined: proj half-chains of block tb+1 and
    output-projection units of block qb-1 are interleaved into the
    ACT-paced scores stretch one unit per accumulated lag quantum.
  - tail: the last block's output stores go out per-512-col chunk right
    after each chunk's psum drain.
"""
import sys

sys.path.insert(0, "/opt/trn_rl_repo")

import math
import numpy as np

B, T, E = 2, 2048, 2048
G, QPG, D = 4, 4, 128
NQ = QPG * D          # 512 q columns per group
NKV = 2 * D           # 256 kv columns per group
TT = T // 128         # 16 t-tiles
TB = T // 512         # 4 t-blocks
NE = E // 128         # 16 e-tiles
ISD = 1.0 / math.sqrt(D)

# wqkv column order: [q0 | q1 | k | v | q2 | q3]
WCOL = {0: 0, 1: 128, 4: 256, 5: 384, 2: 512, 3: 640}

_compiled = None


def _build():
    from collections import deque
    from concourse import bacc, tile, mybir

    f32 = mybir.dt.float32
    bf16 = mybir.dt.bfloat16
    ADD = mybir.AluOpType.add
    MULT = mybir.AluOpType.mult
    EXP = mybir.ActivationFunctionType.Exp

    nc = bacc.Bacc("TRN2", target_bir_lowering=False, debug=False, num_devices=8)

    xt_d = nc.dram_tensor("xt", [128, NE, T], bf16, kind="ExternalInput")    # x^T packed
    wqkv_d = nc.dram_tensor("wqkv", [128, NE, NQ + NKV], bf16, kind="ExternalInput")
    wo_d = nc.dram_tensor("wo", [NQ, E], bf16, kind="ExternalInput")        # group slice
    pet_d = nc.dram_tensor("pet", [D, T], bf16, kind="ExternalInput")         # pe^T
    bq_d = nc.dram_tensor("bq", [D, QPG], f32, kind="ExternalInput")
    bk_d = nc.dram_tensor("bk", [D, 1], f32, kind="ExternalInput")
    bv_d = nc.dram_tensor("bv", [D, 1], f32, kind="ExternalInput")
    msk_d = nc.dram_tensor("msk", [4, 128, 512], bf16, kind="ExternalInput")
    idb_d = nc.dram_tensor("idb", [128, 128], bf16, kind="ExternalInput")
    ones_d = nc.dram_tensor("ones", [128, 128], bf16, kind="ExternalInput")
    out_d = nc.dram_tensor("out", [T, E], bf16, kind="ExternalOutput")

    with tile.TileContext(nc) as tc:
        with (
            tc.tile_pool(name="main", bufs=1) as pp,
            tc.tile_pool(name="ps", bufs=1, space="PSUM") as ps,
        ):
            # ---- persistent constants (gpsimd DMA queue; the phase-1 weight
            # stream is split over three queues so these never delay it) ----
            bv = pp.tile([D, 1], f32)
            nc.gpsimd.dma_start(bv[:], bv_d[:])
            bq = pp.tile([D, QPG], f32)
            nc.gpsimd.dma_start(bq[:], bq_d[:])
            bk = pp.tile([D, 1], f32)
            nc.gpsimd.dma_start(bk[:], bk_d[:])
            idb = pp.tile([128, 128], bf16)
            nc.gpsimd.dma_start(idb[:], idb_d[:])
            ones = pp.tile([128, 128], bf16)
            nc.gpsimd.dma_start(ones[:], ones_d[:])
            pet = pp.tile([D, T], bf16)
            nc.scalar.dma_start(pet[:], pet_d[:])   # needed by first drains
            msk = [pp.tile([128, 512], bf16, name=f"msk{j}", tag=f"msk{j}") for j in range(4)]
            wo_sb = [pp.tile([128, E], bf16, name=f"wo{h}", tag=f"wo{h}") for h in range(QPG)]

            # ---- persistent activations ----
            qt = [pp.tile([128, T], bf16, name=f"qt{h}", tag=f"qt{h}") for h in range(QPG)]
            kt = pp.tile([128, T], bf16)
            at = [pp.tile([128, T], bf16, name=f"at{h}", tag=f"at{h}") for h in range(QPG)]
            vxall = pp.tile([128, T], bf16, name="vxall", tag="vxall")

            # ---- phase-1 weights + x^T stream; the [q0,q1,k,v] and [q2,q3]
            # column halves live in separate tiles so the late second-half
            # DMA can never create a false hazard against first-half reads ----
            wqkv_sb = [pp.tile([128, 4, 512], bf16, name=f"wqkv{g}", tag=f"wqkv{g}")
                       for g in range(4)]
            wqkv2_sb = [pp.tile([128, 4, 256], bf16, name=f"wqkv2_{g}", tag=f"wqkv2_{g}")
                        for g in range(4)]
            xt_t = [[None] * 4 for _ in range(TB)]

            def emit_xt_dma(tb):
                ts = slice(tb * 512, (tb + 1) * 512)
                for g in range(4):
                    xt_t[tb][g] = pp.tile([128, 4, 512], bf16, name="xt", tag="xt", bufs=6)
                    eng = nc.sync if g % 2 == 0 else nc.gpsimd
                    eng.dma_start(xt_t[tb][g][:], xt_d[:, 4 * g:4 * g + 4, ts])

            # block-0 feed is latency-critical: per-e DMAs of the [q0,q1,k,v]
            # weight half plus x^T, spread over three engine queues in e order
            # so the interleaved chains below consume tiles at delivery rate
            for g in range(4):
                xt_t[0][g] = pp.tile([128, 4, 512], bf16, name="xt", tag="xt", bufs=6)
            qs3 = [nc.sync, nc.scalar, nc.gpsimd]
            for e in range(NE):
                g, i = divmod(e, 4)
                qs3[(2 * e) % 3].dma_start(wqkv_sb[g][:, i, :], wqkv_d[:, e, 0:512])
                qs3[(2 * e + 1) % 3].dma_start(xt_t[0][g][:, i, :], xt_d[:, e, 0:512])
            # q2/q3 weight halves arrive during pair-(0,1) scores
            for g in range(4):
                qs3[g % 3].dma_start(wqkv2_sb[g][:], wqkv_d[:, 4 * g:4 * g + 4, 512:768])
            for j in range(4):
                nc.gpsimd.dma_start(msk[j][:], msk_d[j])
            for h in range(QPG):
                nc.scalar.dma_start(wo_sb[h][:], wo_d[h * 128:(h + 1) * 128, :])

            def wsel(e, c):
                g, i = divmod(e, 4)
                if c in (2, 3):
                    return wqkv2_sb[g][:, i, (c - 2) * 128:(c - 1) * 128]
                return wqkv_sb[g][:, i, WCOL[c]:WCOL[c] + 128]

            def v_bias(ps_t):
                vtb = pp.tile([128, 512], bf16, name="vtb", tag="vtb", bufs=2)
                nc.vector.tensor_tensor(vtb[:], ps_t[:],
                                        bv[:].to_broadcast([128, 512]), ADD)
                return vtb

            def v_finish(ts, vtb):
                vtp = ps.tile([128, 512], bf16, name="vtp", tag="vtp", bufs=1)
                for i in range(4):
                    nc.tensor.transpose(vtp[:, i * 128:(i + 1) * 128],
                                        vtb[:, i * 128:(i + 1) * 128], idb[:])
                nc.vector.tensor_copy(vxall[:, ts], vtp[:])

            def proj_drain(c, ts, ps_t):
                if c < 4:
                    nc.vector.tensor_tensor(ps_t[:], ps_t[:],
                                            bq[:, c:c + 1].to_broadcast([128, 512]), ADD)
                    nc.vector.tensor_tensor(qt[c][:, ts], ps_t[:], pet[:, ts], ADD)
                elif c == 4:
                    nc.vector.tensor_tensor(ps_t[:], ps_t[:],
                                            bk[:].to_broadcast([128, 512]), ADD)
                    nc.vector.tensor_tensor(kt[:, ts], ps_t[:], pet[:, ts], ADD)
                else:
                    v_finish(ts, v_bias(ps_t))

            # ---- unit generators; units are (cost_ns, fn) ----
            def proj_units(tb, c):
                # two half-chain units + a drain emitted one unit late so the
                # completed psum never blocks masks/exps behind it on DVE
                ts = slice(tb * 512, (tb + 1) * 512)
                state = {}

                def half_a():
                    ps_t = ps.tile([128, 512], f32, name="big", tag="big", bufs=3)
                    state["ps"] = ps_t
                    for e in range(8):
                        nc.tensor.matmul(ps_t[:], wsel(e, c), xt_t[tb][e // 4][:, e % 4, :],
                                         start=(e == 0), stop=False)

                def half_b():
                    ps_t = state["ps"]
                    for e in range(8, NE):
                        nc.tensor.matmul(ps_t[:], wsel(e, c), xt_t[tb][e // 4][:, e % 4, :],
                                         start=False, stop=(e == NE - 1))

                def drain():
                    proj_drain(c, ts, state["ps"])

                return [(1800, half_a), (1800, half_b), (300, drain)]

            def oproj_units(ti, last=False):
                state = {}

                def alloc():
                    state["o"] = pp.tile([128, E], bf16, name="osb", tag="osb", bufs=2)

                state["pend"] = []

                def drain_oldest():
                    # drains run late and always on DVE: by emission time the
                    # psum is long finished, so neither the exp stream (ACT)
                    # nor the mask stream (DVE) ever waits behind it. Final
                    # block: store each chunk right after its drain so the
                    # last DMA starts as early as possible.
                    eo, w_ps = state["pend"].pop(0)
                    nc.vector.tensor_copy(state["o"][:, eo * 512:(eo + 1) * 512], w_ps[:])
                    if last:
                        eng = nc.sync if eo % 2 == 0 else nc.scalar
                        eng.dma_start(out_d[ti * 128:(ti + 1) * 128, eo * 512:(eo + 1) * 512],
                                      state["o"][:, eo * 512:(eo + 1) * 512])

                units = [(0, alloc)]
                for eo in range(4):
                    def one(eo=eo):
                        if len(state["pend"]) >= (1 if last else 2):
                            drain_oldest()
                        w_ps = ps.tile([128, 512], f32, name="big", tag="big", bufs=3)
                        for h in range(QPG):
                            nc.tensor.matmul(
                                w_ps[:], at[h][:, ti * 128:(ti + 1) * 128],
                                wo_sb[h][:, eo * 512:(eo + 1) * 512],
                                start=(h == 0), stop=(h == QPG - 1),
                            )
                        state["pend"].append((eo, w_ps))
                    units.append((1000, one))

                def store():
                    while state["pend"]:
                        drain_oldest()
                    if not last:
                        eng = nc.scalar if ti % 2 else nc.sync
                        eng.dma_start(out_d[ti * 128:(ti + 1) * 128, :], state["o"][:])
                units.append((300, store))
                return units

            # two filler queues: proj has a deadline (before next block's
            # scores), oproj is slack-filled
            fill_proj = deque()
            fill_oproj = deque()
            lag = [0]

            def absorb(extra):
                lag[0] += extra
                while lag[0] > 0 and (fill_proj or fill_oproj):
                    q = fill_proj if fill_proj else fill_oproj
                    cost, fn = q.popleft()
                    fn()
                    lag[0] -= cost

            # ---- phase-2 per block qb, heads in pairs. Diagonal tiles are
            # column-restricted to [128j, 512). Denominators: DVE-accumulated
            # R, partition-summed by one ones-stationary matmul per head ----
            def phase2_pair(qb, pair):
                qs = slice(qb * 512, (qb + 1) * 512)
                nkt = 4 * qb + 4
                pts = {h: [] for h in pair}
                R = {h: pp.tile([128, 512], bf16, name=f"R{h}", tag=f"R{h}", bufs=2)
                     for h in pair}
                for tk in range(nkt):
                    j = tk - 4 * qb
                    lo = max(0, 128 * j)          # first unmasked col of block
                    w = 512 - lo
                    sl = slice(lo, 512)
                    for h in pair:
                        s_ps = ps.tile([128, 512], f32, name="s", tag="s", bufs=2)
                        nc.tensor.matmul(s_ps[:, sl], kt[:, tk * 128:(tk + 1) * 128],
                                         qt[h][:, qb * 512 + lo:qb * 512 + 512],
                                         start=True, stop=True)
                        p_t = pp.tile([128, 512], bf16, name="pt", tag="pt", bufs=40)
                        nc.scalar.activation(p_t[:, sl], s_ps[:, sl], EXP, scale=ISD)
                        if j >= 0:
                            nc.vector.tensor_tensor(p_t[:, sl], p_t[:, sl], msk[j][:, sl], MULT)
                        pts[h].append((p_t, sl))
                        absorb(160 + (309 * w) // 512)
                o_ps = {h: ps.tile([128, 512], f32, name="o", tag="o", bufs=2)
                        for h in pair}
                for tk in range(nkt):
                    for h in pair:
                        p_t, sl = pts[h][tk]
                        nc.tensor.matmul(o_ps[h][:, sl], vxall[:, tk * 128:(tk + 1) * 128],
                                         p_t[:, sl], start=(tk == 0), stop=(tk == nkt - 1),
                                         skip_group_check=True)
                        # R accumulation rides the PV stretch, when DVE is
                        # otherwise idle
                        if tk == 0:
                            nc.vector.tensor_copy(R[h][:], p_t[:])
                        else:
                            nc.vector.tensor_tensor(R[h][:, sl], R[h][:, sl], p_t[:, sl], ADD)
                for h in pair:
                    rs = ps.tile([128, 512], f32, name="s", tag="s", bufs=2)
                    nc.tensor.matmul(rs[:], ones[:], R[h][:], start=True, stop=True)
                    r = pp.tile([128, 512], f32, name=f"rcp{h}", tag=f"rcp{h}", bufs=2)
                    nc.vector.reciprocal_approx_fast(r[:], rs[:])
                    nc.vector.tensor_tensor(at[h][:, qs], o_ps[h][:], r[:], MULT)

            # ---- drive ----
            # block-0 is DMA-paced: interleave the chains pair (0,1) needs
            # (q0, q1, k, v) e-outer so the PE tracks the DMA delivery
            # frontier; q2/q3 become fillers for pair (0,1)'s scores
            ch_ps = {}
            for c in (0, 1, 4):
                ch_ps[c] = ps.tile([128, 512], f32, name="big", tag="big", bufs=3)
            ch_ps[5] = ps.tile([128, 512], f32, name="s", tag="s", bufs=2)
            for e in range(NE):
                for c in (0, 1, 4, 5):
                    nc.tensor.matmul(ch_ps[c][:], wsel(e, c), xt_t[0][e // 4][:, e % 4, :],
                                     start=(e == 0), stop=(e == NE - 1))
            # drains in score-dependency order (q0, k, q1): the first scores
            # can start after fewer DVE ops. v gets its bias here (frees the
            # s-pool psum) but its transposes ride the first filler slot
            for c in (0, 4, 1):
                proj_drain(c, slice(0, 512), ch_ps[c])
            vtb0 = v_bias(ch_ps[5])

            def v_finish0():
                v_finish(slice(0, 512), vtb0)

            for qb in range(TB):
                if qb == 0:
                    tb0_rest = [(900, v_finish0)] + proj_units(0, 2) + proj_units(0, 3)
                    tb0_fns = {fn for _, fn in tb0_rest}
                    fill_proj.extend(tb0_rest)
                if qb + 1 < TB:
                    emit_xt_dma(qb + 1)
                    for c in range(6):
                        fill_proj.extend(proj_units(qb + 1, c))
                for pair in ((0, 1), (2, 3)):
                    if qb == 0 and pair == (2, 3):
                        # q2/q3 (and v) of block 0 must land before this
                        while fill_proj and fill_proj[0][1] in tb0_fns:
                            fill_proj.popleft()[1]()
                    phase2_pair(qb, pair)
                    if pair == (2, 3):
                        for ti in range(4 * qb, 4 * qb + 4):
                            fill_oproj.extend(oproj_units(ti, last=(qb == TB - 1)))
                # proj chains for tb=qb+1 must land before scores(qb+1)
                while fill_proj:
                    fill_proj.popleft()[1]()
            while fill_oproj:
                fill_oproj.popleft()[1]()

    nc.compile()
    return nc


def _get_compiled():
    global _compiled
    if _compiled is None:
        _compiled = _build()
    return _compiled


def _host_inputs(x, wq, bq, wkv, bkv, wo):
    import jax.numpy as jnp

    def to_bf16(a):
        return np.asarray(jnp.asarray(a, dtype=jnp.bfloat16))

    pos = np.arange(T, dtype=np.float32)[:, None]
    i = np.arange(0, D, 2, dtype=np.float32)
    inv = np.exp(-(np.log(10000.0) * i / D))
    ang = pos * inv
    pe = np.zeros((T, D), np.float32)
    pe[:, 0::2] = np.sin(ang)
    pe[:, 1::2] = np.cos(ang)
    pet = np.ascontiguousarray(pe.T)                       # [D, T]

    # causal masks for the 4 diagonal tiles of a 512-wide tq block:
    # mask_j[p, c] = 1 if c >= 128*j + p
    c = np.arange(512)[None, :]
    p = np.arange(128)[:, None]
    msk = to_bf16(np.stack([(c >= 128 * j + p) for j in range(4)]).astype(np.float32))
    idb = to_bf16(np.eye(128, dtype=np.float32))
    ones = to_bf16(np.ones((128, 128), dtype=np.float32))

    xts = [to_bf16(np.ascontiguousarray(
        x[b].T.reshape(NE, 128, T).transpose(1, 0, 2))) for b in range(B)]
    in_maps = []
    for core in range(8):
        b, g = divmod(core, G)
        bq_g = bq[g * NQ:(g + 1) * NQ].reshape(QPG, D)     # [h, d]
        qh = [wq[:, g * NQ + h * 128:g * NQ + (h + 1) * 128] for h in range(QPG)]
        kcol = wkv[:, g * NKV:g * NKV + D]
        vcol = wkv[:, g * NKV + D:(g + 1) * NKV]
        # column order [q0, q1, k, v, q2, q3] to match WCOL
        wqkv = np.concatenate([qh[0], qh[1], kcol, vcol, qh[2], qh[3]], axis=1)
        in_maps.append({
            "xt": xts[b],
            "wqkv": to_bf16(np.ascontiguousarray(
                wqkv.reshape(NE, 128, NQ + NKV).transpose(1, 0, 2))),
            "wo": to_bf16(wo[g * NQ:(g + 1) * NQ, :]),
            "pet": to_bf16(pet),
            "bq": np.ascontiguousarray(bq_g.T).astype(np.float32),
            "bk": np.ascontiguousarray(
                bkv[g * NKV:g * NKV + D].reshape(D, 1)).astype(np.float32),
            "bv": np.ascontiguousarray(
                bkv[g * NKV + D:(g + 1) * NKV].reshape(D, 1)).astype(np.float32),
            "msk": msk,
            "idb": idb,
            "ones": ones,
        })
    return in_maps


def run(x, wq, bq, wkv, bkv, wo, trace=False):
    from concourse.bass_utils import run_bass_kernel_spmd

    nc = _get_compiled()
    in_maps = _host_inputs(
        np.asarray(x, np.float32), np.asarray(wq, np.float32),
        np.asarray(bq, np.float32), np.asarray(wkv, np.float32),
        np.asarray(bkv, np.float32), np.asarray(wo, np.float32),
    )
    res = run_bass_kernel_spmd(nc, in_maps, core_ids=list(range(8)), trace=trace)
    out = np.zeros((B, T, E), np.float32)
    for core in range(8):
        b = core // G
        out[b] += np.asarray(res.results[core]["out"], dtype=np.float32)
    return out, res


def kernel(x, wq, bq, wkv, bkv, wo):
    out, _ = run(x, wq, bq, wkv, bkv, wo, trace=False)
    return out


# revision 28
# speedup vs baseline: 1.0252x; 1.0116x over previous
"""GQA kernel for Trainium2: B=2,T=2048,E=2048,G=4,QPG=4,D=128, causal + sinusoidal PE.

Sharding: one core per (batch, kv-group) pair = 2*4 = 8 cores.
Each core computes q/k/v projections for its group, attention for its 4 query
heads, and a partial output projection (its group's 512 columns of wo);
partials are summed on the host.

v3 layout strategy (per core), all matmul operands bf16:
  - wqkv columns are host-reordered to [q0,q1,k,v,q2,q3] and DMA'd in two
    halves (separate tiles) so everything pair (0,1) needs lands first;
    block-0's four critical chains (q0,q1,k,v) are interleaved e-outer so
    the PE consumes tiles at DMA delivery rate.
  - projections: weight tile stationary, x^T streams; per (tb, chain) one
    512-col psum chain over 16 e-tiles.
  - scores S^T[tk, tq]: kt tile stationary, q^T streams; exp'd on ACT into
    bf16 P^T tiles. Diagonal tiles are column-restricted to the unmasked
    range [128j, 512) for the matmul, exp, mask, R-add and PV.
  - PV: V tile [tk, d] stationary, P^T streams, O^T[d, tq] in psum.
  - softmax denominators: R = sum_tk P^T accumulated on DVE, then ONE PE
    matmul against an all-ones stationary matrix replicates the column sums
    to all partitions (replaces the 3.6us gpsimd partition_all_reduce that
    head-of-line blocked the DVE queue at block boundaries); reciprocal
    multiplies O^T during the psum drain.
  - phases are software-pipelined: proj half-chains of block tb+1 and
    output-projection units of block qb-1 are interleaved into the
    ACT-paced scores stretch one unit per accumulated lag quantum.
  - tail: the last block's output stores go out per-512-col chunk right
    after each chunk's psum drain.
"""
import sys

sys.path.insert(0, "/opt/trn_rl_repo")

import math
import numpy as np

B, T, E = 2, 2048, 2048
G, QPG, D = 4, 4, 128
NQ = QPG * D          # 512 q columns per group
NKV = 2 * D           # 256 kv columns per group
TT = T // 128         # 16 t-tiles
TB = T // 512         # 4 t-blocks
NE = E // 128         # 16 e-tiles
ISD = 1.0 / math.sqrt(D)

# wqkv column order: [q0 | q1 | k | v | q2 | q3]
WCOL = {0: 0, 1: 128, 4: 256, 5: 384, 2: 512, 3: 640}

_compiled = None


def _build():
    from collections import deque
    from concourse import bacc, tile, mybir

    f32 = mybir.dt.float32
    bf16 = mybir.dt.bfloat16
    ADD = mybir.AluOpType.add
    MULT = mybir.AluOpType.mult
    EXP = mybir.ActivationFunctionType.Exp

    nc = bacc.Bacc("TRN2", target_bir_lowering=False, debug=False, num_devices=8)

    xt_d = nc.dram_tensor("xt", [128, NE, T], bf16, kind="ExternalInput")    # x^T packed
    wqkv_d = nc.dram_tensor("wqkv", [128, NE, NQ + NKV], bf16, kind="ExternalInput")
    wo_d = nc.dram_tensor("wo", [NQ, E], bf16, kind="ExternalInput")        # group slice
    pet_d = nc.dram_tensor("pet", [D, T], bf16, kind="ExternalInput")         # pe^T
    bq_d = nc.dram_tensor("bq", [D, QPG], f32, kind="ExternalInput")
    bk_d = nc.dram_tensor("bk", [D, 1], f32, kind="ExternalInput")
    bv_d = nc.dram_tensor("bv", [D, 1], f32, kind="ExternalInput")
    msk_d = nc.dram_tensor("msk", [4, 128, 512], bf16, kind="ExternalInput")
    idb_d = nc.dram_tensor("idb", [128, 128], bf16, kind="ExternalInput")
    ones_d = nc.dram_tensor("ones", [128, 128], bf16, kind="ExternalInput")
    out_d = nc.dram_tensor("out", [T, E], bf16, kind="ExternalOutput")

    with tile.TileContext(nc) as tc:
        with (
            tc.tile_pool(name="main", bufs=1) as pp,
            tc.tile_pool(name="ps", bufs=1, space="PSUM") as ps,
        ):
            # ---- persistent constants (gpsimd DMA queue; the phase-1 weight
            # stream is split over three queues so these never delay it) ----
            bv = pp.tile([D, 1], f32)
            nc.gpsimd.dma_start(bv[:], bv_d[:])
            bq = pp.tile([D, QPG], f32)
            nc.gpsimd.dma_start(bq[:], bq_d[:])
            bk = pp.tile([D, 1], f32)
            nc.gpsimd.dma_start(bk[:], bk_d[:])
            idb = pp.tile([128, 128], bf16)
            nc.gpsimd.dma_start(idb[:], idb_d[:])
            ones = pp.tile([128, 128], bf16)
            nc.gpsimd.dma_start(ones[:], ones_d[:])
            pet = pp.tile([D, T], bf16)
            nc.scalar.dma_start(pet[:], pet_d[:])   # needed by first drains
            msk = [pp.tile([128, 512], bf16, name=f"msk{j}", tag=f"msk{j}") for j in range(4)]
            wo_sb = [pp.tile([128, E], bf16, name=f"wo{h}", tag=f"wo{h}") for h in range(QPG)]

            # ---- persistent activations ----
            qt = [pp.tile([128, T], bf16, name=f"qt{h}", tag=f"qt{h}") for h in range(QPG)]
            kt = pp.tile([128, T], bf16)
            at = [pp.tile([128, T], bf16, name=f"at{h}", tag=f"at{h}") for h in range(QPG)]
            vxall = pp.tile([128, T], bf16, name="vxall", tag="vxall")

            # ---- phase-1 weights + x^T stream; the [q0,q1,k,v] and [q2,q3]
            # column halves live in separate tiles so the late second-half
            # DMA can never create a false hazard against first-half reads ----
            wqkv_sb = [pp.tile([128, 4, 512], bf16, name=f"wqkv{g}", tag=f"wqkv{g}")
                       for g in range(4)]
            wqkv2_sb = [pp.tile([128, 4, 256], bf16, name=f"wqkv2_{g}", tag=f"wqkv2_{g}")
                        for g in range(4)]
            xt_t = [[None] * 4 for _ in range(TB)]

            def emit_xt_dma(tb):
                ts = slice(tb * 512, (tb + 1) * 512)
                for g in range(4):
                    xt_t[tb][g] = pp.tile([128, 4, 512], bf16, name="xt", tag="xt", bufs=6)
                    eng = nc.sync if g % 2 == 0 else nc.gpsimd
                    eng.dma_start(xt_t[tb][g][:], xt_d[:, 4 * g:4 * g + 4, ts])

            # block-0 feed is latency-critical: per-e DMAs of the [q0,q1,k,v]
            # weight half plus x^T, spread over three engine queues in e order
            # so the interleaved chains below consume tiles at delivery rate
            for g in range(4):
                xt_t[0][g] = pp.tile([128, 4, 512], bf16, name="xt", tag="xt", bufs=6)
            qs3 = [nc.sync, nc.scalar, nc.gpsimd]
            for e in range(NE):
                g, i = divmod(e, 4)
                qs3[(2 * e) % 3].dma_start(wqkv_sb[g][:, i, :], wqkv_d[:, e, 0:512])
                qs3[(2 * e + 1) % 3].dma_start(xt_t[0][g][:, i, :], xt_d[:, e, 0:512])
            # q2/q3 weight halves arrive during pair-(0,1) scores
            for g in range(4):
                qs3[g % 3].dma_start(wqkv2_sb[g][:], wqkv_d[:, 4 * g:4 * g + 4, 512:768])
            for j in range(4):
                nc.gpsimd.dma_start(msk[j][:], msk_d[j])
            for h in range(QPG):
                nc.scalar.dma_start(wo_sb[h][:], wo_d[h * 128:(h + 1) * 128, :])

            def wsel(e, c):
                g, i = divmod(e, 4)
                if c in (2, 3):
                    return wqkv2_sb[g][:, i, (c - 2) * 128:(c - 1) * 128]
                return wqkv_sb[g][:, i, WCOL[c]:WCOL[c] + 128]

            def v_bias(ps_t):
                vtb = pp.tile([128, 512], bf16, name="vtb", tag="vtb", bufs=2)
                nc.vector.tensor_tensor(vtb[:], ps_t[:],
                                        bv[:].to_broadcast([128, 512]), ADD)
                return vtb

            def v_finish(ts, vtb):
                vtp = ps.tile([128, 512], bf16, name="vtp", tag="vtp", bufs=1)
                for i in range(4):
                    nc.tensor.transpose(vtp[:, i * 128:(i + 1) * 128],
                                        vtb[:, i * 128:(i + 1) * 128], idb[:])
                nc.vector.tensor_copy(vxall[:, ts], vtp[:])

            def proj_drain(c, ts, ps_t):
                if c < 4:
                    nc.vector.tensor_tensor(ps_t[:], ps_t[:],
                                            bq[:, c:c + 1].to_broadcast([128, 512]), ADD)
                    nc.vector.tensor_tensor(qt[c][:, ts], ps_t[:], pet[:, ts], ADD)
                elif c == 4:
                    nc.vector.tensor_tensor(ps_t[:], ps_t[:],
                                            bk[:].to_broadcast([128, 512]), ADD)
                    nc.vector.tensor_tensor(kt[:, ts], ps_t[:], pet[:, ts], ADD)
                else:
                    v_finish(ts, v_bias(ps_t))

            # ---- unit generators; units are (cost_ns, fn) ----
            def proj_units(tb, c):
                # two half-chain units + a drain emitted one unit late so the
                # completed psum never blocks masks/exps behind it on DVE
                ts = slice(tb * 512, (tb + 1) * 512)
                state = {}

                def half_a():
                    ps_t = ps.tile([128, 512], f32, name="big", tag="big", bufs=3)
                    state["ps"] = ps_t
                    for e in range(8):
                        nc.tensor.matmul(ps_t[:], wsel(e, c), xt_t[tb][e // 4][:, e % 4, :],
                                         start=(e == 0), stop=False)

                def half_b():
                    ps_t = state["ps"]
                    for e in range(8, NE):
                        nc.tensor.matmul(ps_t[:], wsel(e, c), xt_t[tb][e // 4][:, e % 4, :],
                                         start=False, stop=(e == NE - 1))

                def drain():
                    proj_drain(c, ts, state["ps"])

                return [(1800, half_a), (1800, half_b), (300, drain)]

            def oproj_units(ti, last=False):
                state = {}

                def alloc():
                    state["o"] = pp.tile([128, E], bf16, name="osb", tag="osb", bufs=2)

                state["pend"] = []

                def drain_oldest():
                    # drains run late and always on DVE: by emission time the
                    # psum is long finished, so neither the exp stream (ACT)
                    # nor the mask stream (DVE) ever waits behind it. Final
                    # block: store each chunk right after its drain so the
                    # last DMA starts as early as possible.
                    eo, w_ps = state["pend"].pop(0)
                    nc.vector.tensor_copy(state["o"][:, eo * 512:(eo + 1) * 512], w_ps[:])
                    if last:
                        eng = nc.sync if eo % 2 == 0 else nc.scalar
                        eng.dma_start(out_d[ti * 128:(ti + 1) * 128, eo * 512:(eo + 1) * 512],
                                      state["o"][:, eo * 512:(eo + 1) * 512])

                units = [(0, alloc)]
                for eo in range(4):
                    def one(eo=eo):
                        if len(state["pend"]) >= (1 if last else 2):
                            drain_oldest()
                        w_ps = ps.tile([128, 512], f32, name="big", tag="big", bufs=3)
                        for h in range(QPG):
                            nc.tensor.matmul(
                                w_ps[:], at[h][:, ti * 128:(ti + 1) * 128],
                                wo_sb[h][:, eo * 512:(eo + 1) * 512],
                                start=(h == 0), stop=(h == QPG - 1),
                            )
                        state["pend"].append((eo, w_ps))
                    units.append((1000, one))

                def store():
                    while state["pend"]:
                        drain_oldest()
                    if not last:
                        eng = nc.scalar if ti % 2 else nc.sync
                        eng.dma_start(out_d[ti * 128:(ti + 1) * 128, :], state["o"][:])
                units.append((300, store))
                return units

            # two filler queues: proj has a deadline (before next block's
            # scores), oproj is slack-filled
            fill_proj = deque()
            fill_oproj = deque()
            lag = [0]

            def absorb(extra):
                lag[0] += extra
                while lag[0] > 0 and (fill_proj or fill_oproj):
                    q = fill_proj if fill_proj else fill_oproj
                    cost, fn = q.popleft()
                    fn()
                    lag[0] -= cost

            # ---- phase-2 per block qb, heads in pairs. Diagonal tiles are
            # column-restricted to [128j, 512). Denominators: DVE-accumulated
            # R, partition-summed by one ones-stationary matmul per head ----
            def phase2_pair(qb, pair):
                qs = slice(qb * 512, (qb + 1) * 512)
                nkt = 4 * qb + 4
                pts = {h: [] for h in pair}
                R = {h: pp.tile([128, 512], bf16, name=f"R{h}", tag=f"R{h}", bufs=2)
                     for h in pair}
                for tk in range(nkt):
                    j = tk - 4 * qb
                    lo = max(0, 128 * j)          # first unmasked col of block
                    w = 512 - lo
                    sl = slice(lo, 512)
                    for h in pair:
                        s_ps = ps.tile([128, 512], f32, name="s", tag="s", bufs=2)
                        nc.tensor.matmul(s_ps[:, sl], kt[:, tk * 128:(tk + 1) * 128],
                                         qt[h][:, qb * 512 + lo:qb * 512 + 512],
                                         start=True, stop=True)
                        p_t = pp.tile([128, 512], bf16, name="pt", tag="pt", bufs=40)
                        nc.scalar.activation(p_t[:, sl], s_ps[:, sl], EXP, scale=ISD)
                        if j >= 0:
                            nc.vector.tensor_tensor(p_t[:, sl], p_t[:, sl], msk[j][:, sl], MULT)
                        pts[h].append((p_t, sl))
                        absorb(160 + (309 * w) // 512)
                o_ps = {h: ps.tile([128, 512], f32, name="o", tag="o", bufs=2)
                        for h in pair}
                for tk in range(nkt):
                    for h in pair:
                        p_t, sl = pts[h][tk]
                        nc.tensor.matmul(o_ps[h][:, sl], vxall[:, tk * 128:(tk + 1) * 128],
                                         p_t[:, sl], start=(tk == 0), stop=(tk == nkt - 1),
                                         skip_group_check=True)
                        # R accumulation rides the PV stretch, when DVE is
                        # otherwise idle
                        if tk == 0:
                            nc.vector.tensor_copy(R[h][:], p_t[:])
                        else:
                            nc.vector.tensor_tensor(R[h][:, sl], R[h][:, sl], p_t[:, sl], ADD)
                for h in pair:
                    rs = ps.tile([128, 512], f32, name="s", tag="s", bufs=2)
                    nc.tensor.matmul(rs[:], ones[:], R[h][:], start=True, stop=True)
                    r = pp.tile([128, 512], f32, name=f"rcp{h}", tag=f"rcp{h}", bufs=2)
                    nc.vector.reciprocal_approx_fast(r[:], rs[:])
                    nc.vector.tensor_tensor(at[h][:, qs], o_ps[h][:], r[:], MULT)

            # ---- drive ----
            # block-0 is DMA-paced: interleave the chains pair (0,1) needs
            # (q0, q1, k, v) e-outer so the PE tracks the DMA delivery
            # frontier; q2/q3 become fillers for pair (0,1)'s scores
            ch_ps = {}
            for c in (0, 1, 4):
                ch_ps[c] = ps.tile([128, 512], f32, name="big", tag="big", bufs=3)
            ch_ps[5] = ps.tile([128, 512], f32, name="s", tag="s", bufs=2)
            for e in range(NE):
                for c in (0, 1, 4, 5):
                    nc.tensor.matmul(ch_ps[c][:], wsel(e, c), xt_t[0][e // 4][:, e % 4, :],
                                     start=(e == 0), stop=(e == NE - 1))
            for c in (0, 1, 4, 5):
                proj_drain(c, slice(0, 512), ch_ps[c])

            for qb in range(TB):
                if qb == 0:
                    tb0_rest = proj_units(0, 2) + proj_units(0, 3)
                    tb0_fns = {fn for _, fn in tb0_rest}
                    fill_proj.extend(tb0_rest)
                if qb + 1 < TB:
                    emit_xt_dma(qb + 1)
                    for c in range(6):
                        fill_proj.extend(proj_units(qb + 1, c))
                for pair in ((0, 1), (2, 3)):
                    if qb == 0 and pair == (2, 3):
                        # q2/q3 (and v) of block 0 must land before this
                        while fill_proj and fill_proj[0][1] in tb0_fns:
                            fill_proj.popleft()[1]()
                    phase2_pair(qb, pair)
                    if pair == (2, 3):
                        for ti in range(4 * qb, 4 * qb + 4):
                            fill_oproj.extend(oproj_units(ti, last=(qb == TB - 1)))
                # proj chains for tb=qb+1 must land before scores(qb+1)
                while fill_proj:
                    fill_proj.popleft()[1]()
            while fill_oproj:
                fill_oproj.popleft()[1]()

    nc.compile()
    return nc


def _get_compiled():
    global _compiled
    if _compiled is None:
        _compiled = _build()
    return _compiled


def _host_inputs(x, wq, bq, wkv, bkv, wo):
    import jax.numpy as jnp

    def to_bf16(a):
        return np.asarray(jnp.asarray(a, dtype=jnp.bfloat16))

    pos = np.arange(T, dtype=np.float32)[:, None]
    i = np.arange(0, D, 2, dtype=np.float32)
    inv = np.exp(-(np.log(10000.0) * i / D))
    ang = pos * inv
    pe = np.zeros((T, D), np.float32)
    pe[:, 0::2] = np.sin(ang)
    pe[:, 1::2] = np.cos(ang)
    pet = np.ascontiguousarray(pe.T)                       # [D, T]

    # causal masks for the 4 diagonal tiles of a 512-wide tq block:
    # mask_j[p, c] = 1 if c >= 128*j + p
    c = np.arange(512)[None, :]
    p = np.arange(128)[:, None]
    msk = to_bf16(np.stack([(c >= 128 * j + p) for j in range(4)]).astype(np.float32))
    idb = to_bf16(np.eye(128, dtype=np.float32))
    ones = to_bf16(np.ones((128, 128), dtype=np.float32))

    xts = [to_bf16(np.ascontiguousarray(
        x[b].T.reshape(NE, 128, T).transpose(1, 0, 2))) for b in range(B)]
    in_maps = []
    for core in range(8):
        b, g = divmod(core, G)
        bq_g = bq[g * NQ:(g + 1) * NQ].reshape(QPG, D)     # [h, d]
        qh = [wq[:, g * NQ + h * 128:g * NQ + (h + 1) * 128] for h in range(QPG)]
        kcol = wkv[:, g * NKV:g * NKV + D]
        vcol = wkv[:, g * NKV + D:(g + 1) * NKV]
        # column order [q0, q1, k, v, q2, q3] to match WCOL
        wqkv = np.concatenate([qh[0], qh[1], kcol, vcol, qh[2], qh[3]], axis=1)
        in_maps.append({
            "xt": xts[b],
            "wqkv": to_bf16(np.ascontiguousarray(
                wqkv.reshape(NE, 128, NQ + NKV).transpose(1, 0, 2))),
            "wo": to_bf16(wo[g * NQ:(g + 1) * NQ, :]),
            "pet": to_bf16(pet),
            "bq": np.ascontiguousarray(bq_g.T).astype(np.float32),
            "bk": np.ascontiguousarray(
                bkv[g * NKV:g * NKV + D].reshape(D, 1)).astype(np.float32),
            "bv": np.ascontiguousarray(
                bkv[g * NKV + D:(g + 1) * NKV].reshape(D, 1)).astype(np.float32),
            "msk": msk,
            "idb": idb,
            "ones": ones,
        })
    return in_maps


def run(x, wq, bq, wkv, bkv, wo, trace=False):
    from concourse.bass_utils import run_bass_kernel_spmd

    nc = _get_compiled()
    in_maps = _host_inputs(
        np.asarray(x, np.float32), np.asarray(wq, np.float32),
        np.asarray(bq, np.float32), np.asarray(wkv, np.float32),
        np.asarray(bkv, np.float32), np.asarray(wo, np.float32),
    )
    res = run_bass_kernel_spmd(nc, in_maps, core_ids=list(range(8)), trace=trace)
    out = np.zeros((B, T, E), np.float32)
    for core in range(8):
        b = core // G
        out[b] += np.asarray(res.results[core]["out"], dtype=np.float32)
    return out, res


def kernel(x, wq, bq, wkv, bkv, wo):
    out, _ = run(x, wq, bq, wkv, bkv, wo, trace=False)
    return out
